# revision 66
# baseline (speedup 1.0000x reference)
"""Multi-head self-attention (causal) Trainium2 Bass/Tile kernel, 8-way SPMD.

Sharding: data-parallel over batch (4) x tensor-parallel over heads (2 groups
of 8 heads).  Core c handles batch c//2, head-group c%2.  Each core computes
q/k/v projections for its 512 local features, causal attention for its 8
heads, and a partial o-projection (contraction over its 512 features of the
attention output) giving a full-shape [S, D] partial that the host sums per
batch pair.

All matmul operands are bf16 (fp32 PSUM accumulation); softmax runs without
max-subtraction (scores ~ N(0,1) after the 1/8 scale, no overflow risk), with
exp on the scalar engine and the row-sum folded into the AV matmul via a ones
column appended to V.  Host pre-transposes inputs so no on-chip transposes
are needed:
  qT[e,s]  = wqT.T @ xT        (lhsT=wqT[d,e], rhs=xT[d,s])
  scoresT[sk,sq] = kT.T @ qT   (lhsT=kT[dk,sk], rhs=qT[dk,sq], K=64)
  avT[dk+1,sq]   = vaug.T @ expT  (lhsT=vaug[sk,65], rhs=expT[sk,sq])
  y[s,e]   = outT.T @ woT      (lhsT=outT[d,s], rhs=woT[d,e])

Scheduling: a single global stream over (query-group, head-pair, key-tile)
"attention steps", software-pipelined with the projection / o-projection
matmuls broken into single-matmul filler quanta.  Cost-model cursors for the
PE / ACT / DVE engines decide when the attention stream would stall on the
scalar engine's exp (score-PSUM bank rotation), and filler is injected to
cover the wait; the pair of K=64 score matmuls lands on disjoint PE row
halves (h0/h64) and executes concurrently, and the pacing deliberately
over-budgets them so the PE stays ahead of the exp pipe.  Causality is
exploited at 128-column granularity: diagonal key strips only compute
scores/exp/AV for the query range they can see, with one shared [128,128]
lower-triangular mask for the leading block.  o-proj groups are reserved
(slot gates) for the exp-heavy late query groups, where they are the only
filler left.  Weights are packed host-side so each matrix loads in a few
large DMAs ordered by first need (wq/wk et-major); xT streams token-block-
major across the gpsimd+scalar DMA queues.  y is written back as bf16
partials summed on the host.
"""

from contextlib import ExitStack

import numpy as np
import ml_dtypes

import concourse.bass as bass
import concourse.tile as tile
from concourse import bacc, mybir
from concourse._compat import with_exitstack
from concourse.bass_utils import run_bass_kernel_spmd

B, S, D, H = 4, 2048, 1024, 16
DK = D // H          # 64
E = 512              # local features per core (8 heads)
HL = 8               # local heads
NCORES = 8
NDT = D // 128       # 8 d-tiles
NET = E // 128       # 4 e-tiles
NST = S // 128       # 16 s-tiles
NQG = S // 512       # 4 query groups

F32 = mybir.dt.float32
BF16 = mybir.dt.bfloat16
bf16 = ml_dtypes.bfloat16

# --- scheduler cost model (ns) ------------------------------------------
CYC = 1e9 / 2.4e9          # PE ns per streamed column (bf16)
PE_OVH = 8.0               # per-matmul PE overhead (trace: 220ns/512-col)
ACT_FIX = 220.0            # per-activation fixed cost (trace fit)
ACT_COL = 0.91             # ACT ns per free column (trace fit)
DVE_COL = 1e9 / 0.96e9     # DVE ns per free column (f32 path)
SEM = 100.0                # cross-engine semaphore latency
WARMUP = 22                # PE clock-ramp matmuls at kernel start
MARGIN_BANK = 120.0        # pacing margin for score-PSUM bank reuse
MARGIN_AV = 80.0           # pacing margin for AV reading exp(+mask)
NORM_LAT = 6000.0          # DRAM-bounce + Pool-mul latency for normalize

_compiled = None
last_results = None  # test harness introspection
sched_stats = {}


def _mm_ns(n):
    return n * CYC + PE_OVH


class _Sched:
    """Engine-cursor bookkeeping + filler queue."""

    def __init__(self):
        self.pe = 0.0
        self.act = 0.0
        self.dve = 0.0
        self.groups = []          # [steps:list, fin:callable, gate:callable]
        self.key_idx = {}
        self.cur = 0
        self.active = None
        self.stalled = 0.0
        self.slot_idx = 0
        self.stall_log = []

    # -- filler queue ----------------------------------------------------
    # Strictly ordered groups; pacing may jump over gated (not-yet-eligible)
    # groups, but only ONE jump-ahead group may be in flight at a time so
    # PSUM fill-buffer rotation never interleaves two accumulations.
    def add_group(self, key, steps, fin=None, gate=None):
        self.key_idx[key] = len(self.groups)
        self.groups.append([list(steps), fin, gate])

    def _emit_from(self, idx):
        steps, fin, _ = self.groups[idx]
        if steps:
            steps.pop(0)()
            if not steps and fin is not None:
                fin()
                self.groups[idx][1] = None
            return True
        if fin is not None:
            fin()
            self.groups[idx][1] = None
            return True
        return False

    def _done(self, idx):
        g = self.groups[idx]
        return not g[0] and g[1] is None

    def emit_one(self, pacing=False):
        while self.cur < len(self.groups) and self._done(self.cur):
            self.cur += 1
        if self.active is not None:
            idx = self.active
            if self._done(idx):
                self.active = None
            else:
                self._emit_from(idx)
                if self._done(idx):
                    self.active = None
                return True
        if self.cur >= len(self.groups):
            return False
        steps, fin, gate = self.groups[self.cur]
        if not pacing or gate is None or gate():
            return self._emit_from(self.cur)
        # cur is gated: jump ahead to the first eligible group
        for idx in range(self.cur + 1, len(self.groups)):
            g = self.groups[idx]
            if self._done(idx):
                continue
            if g[2] is not None and not g[2]():
                continue
            self.active = idx
            self._emit_from(idx)
            if self._done(idx):
                self.active = None
            return True
        return False

    def force(self, key):
        idx = self.key_idx[key]
        while self.active is not None:
            self.emit_one()
        while self.cur <= idx and not self._done(idx):
            if not self.emit_one():
                return

    def drain(self):
        while self.emit_one():
            pass

    def pace_to(self, target):
        while self.pe + 1e-9 < target:
            if not self.emit_one(pacing=True):
                self.stalled += target - self.pe
                self.stall_log.append((self.slot_idx, round(self.pe),
                                       round(target - self.pe)))
                self.pe = target
                break


@with_exitstack
def _mhsa_kernel(ctx: ExitStack, tc: tile.TileContext, y, xT, wqT, wkT, wvT,
                 woT, tri):
    nc = tc.nc
    sc = _Sched()

    consts = ctx.enter_context(tc.tile_pool(name="consts", bufs=1))
    ex_pool = ctx.enter_context(tc.tile_pool(name="ex", bufs=6))
    rec_pool = ctx.enter_context(tc.tile_pool(name="rec", bufs=2))
    y_pool = ctx.enter_context(tc.tile_pool(name="ysb", bufs=3))
    ps_pool = ctx.enter_context(tc.tile_pool(name="psmm", bufs=2, space="PSUM"))
    fill_pool = ctx.enter_context(tc.tile_pool(name="psfl", bufs=2, space="PSUM"))
    av_pool = ctx.enter_context(tc.tile_pool(name="psav", bufs=2, space="PSUM"))

    def ctile(shape, dt_, tg):
        return consts.tile(shape, dt_, tag=tg, name=tg)

    # ---- persistent SBUF tiles -------------------------------------------
    # weights are packed host-side as [128, NDT*cols] (dt-major columns) so
    # each matrix loads with a single dma_start
    xT_t = [ctile([128, S], BF16, f"xT{i}") for i in range(NDT)]
    wq_all = ctile([128, NDT * E], BF16, "wq_all")
    wk_all = ctile([128, NDT * E], BF16, "wk_all")
    wv_all = ctile([128, NDT * E], BF16, "wv_all")
    wo_all = ctile([128, NET * D], BF16, "wo_all")
    qT_t = [ctile([128, S], BF16, f"qT{i}") for i in range(NET)]
    kT_t = [ctile([128, S], BF16, f"kT{i}") for i in range(NET)]
    vaug_t = [ctile([128, HL * (DK + 1)], BF16, f"vaug{i}") for i in range(NST)]
    outT_t = [ctile([128, S], BF16, f"outT{i}") for i in range(NET)]
    tri_t = ctile([128, 128], BF16, "tri")

    # wq/wk are packed ET-MAJOR host-side ([128, et*1024 + dt*128 + c]) so
    # the first head-pair's weights arrive as one small early DMA; wv/wo
    # stay dt-major (their consumers need all of them anyway)
    def wq_sl(et, dt):
        return wq_all[:, et * 1024 + dt * 128:et * 1024 + (dt + 1) * 128]

    def wk_sl(et, dt):
        return wk_all[:, et * 1024 + dt * 128:et * 1024 + (dt + 1) * 128]

    wvT_t = [wv_all[:, i * E:(i + 1) * E] for i in range(NDT)]
    woT_t = [wo_all[:, i * D:(i + 1) * D] for i in range(NET)]

    # ---- input DMA, ordered by first need ---------------------------------
    nc.sync.dma_start(out=tri_t, in_=tri)
    for et in range(NET):
        nc.sync.dma_start(out=wq_all[:, et * 1024:(et + 1) * 1024],
                          in_=wqT[:, et * 1024:(et + 1) * 1024])
        nc.sync.dma_start(out=wk_all[:, et * 1024:(et + 1) * 1024],
                          in_=wkT[:, et * 1024:(et + 1) * 1024])
        if et == 0:
            nc.sync.dma_start(out=wv_all, in_=wvT)
    # warm memset goes on gpsimd so it lands before anything else that
    # engine issues; the DVE queue is kept clear for latency-critical masks
    warm = ctile([128, 512], BF16, "warm")
    nc.gpsimd.memset(warm, 0.0)
    # xT token-block tb0 first (gates the first scores), split across the
    # gpsimd and scalar DMA queues with even/odd dt interleaved so the two
    # transfer streams deliver tiles in accumulation order
    for tb in range(2):
        for i in range(NDT):
            q = nc.gpsimd if i % 2 == 0 else nc.scalar
            q.dma_start(out=xT_t[i][:, tb * 512:(tb + 1) * 512],
                        in_=xT[i * 128:(i + 1) * 128, tb * 512:(tb + 1) * 512])
    for i in range(NDT):
        nc.gpsimd.dma_start(
            out=xT_t[i][:, 1024:2048],
            in_=xT[i * 128:(i + 1) * 128, 1024:2048])
    nc.sync.dma_start(out=wo_all, in_=woT)

    # ---- PE warm-up: HAM starts throttled at 1.2 GHz and needs ~3us of
    # sustained matmul activity to release; burn idle DMA-wait time at
    # kernel start so the first real matmuls run at full clock.  One
    # accumulation group: no inter-matmul completion semaphores, so it
    # actually runs back-to-back.  Emitted before the filler groups so the
    # fill-pool rotation order matches emission order.
    wps = fill_pool.tile([128, 512], F32, tag="fl", name="fl")
    for i in range(WARMUP):
        nc.tensor.matmul(wps, lhsT=warm[:, 0:128], rhs=warm,
                         start=(i == 0), stop=(i == WARMUP - 1))
        sc.pe += _mm_ns(512)

    # Softmax denominators bounce through DRAM: DVE can only write at
    # 32-aligned base partitions, and SBUF APs cannot have a step-0
    # partition dim (needed for the broadcast) — DRAM APs can do both.
    sums_dram = nc.dram_tensor("sums_bounce", [NQG, HL, 512], F32).ap()
    rec_dram = nc.dram_tensor("rec_bounce", [NQG, HL, 512], BF16).ap()

    # ones2: selector for the final pair's reciprocal broadcast matmul —
    # bc[j, :] = recb2[0, :] for j<64 (head A) and recb2[32, :] for j>=64
    # (head B).  Rows 0/32 because the DVE can only write at 32-aligned
    # partitions; K padded to 64 (a K=33 matmul wedges the exec unit).
    ones2 = ctile([64, 128], BF16, "ones2")
    nc.vector.memset(ones2, 0.0)
    nc.vector.memset(ones2[0:1, 0:64], 1.0)
    nc.vector.memset(ones2[32:33, 64:128], 1.0)

    # ---- filler groups ----------------------------------------------------
    def qk_group(wsl, dst, et, tb):
        ps = fill_pool.tile([128, 512], F32, tag="fl", name="fl")

        def step(dt_):
            def go():
                nc.tensor.matmul(
                    ps,
                    lhsT=wsl(et, dt_),
                    rhs=xT_t[dt_][:, tb * 512:(tb + 1) * 512],
                    start=(dt_ == 0), stop=(dt_ == NDT - 1),
                )
                sc.pe += _mm_ns(512)
            return go

        def fin():
            # Pool can't read PSUM; scalar Copy keeps the DVE queue clear
            # for the latency-critical masks
            nc.scalar.activation(
                out=dst[et][:, tb * 512:(tb + 1) * 512], in_=ps,
                func=mybir.ActivationFunctionType.Copy)
            sc.act = max(sc.act, sc.pe + SEM) + ACT_FIX + 512 * ACT_COL
        return [step(d) for d in range(NDT)], fin

    def v_group(st):
        ps = fill_pool.tile([128, 512], F32, tag="fl", name="fl")

        def step(dt_):
            def go():
                if dt_ == 0:
                    # lazy memset: keeps the DVE queue clear until needed
                    nc.vector.memset(vaug_t[st], 1.0)
                    sc.dve += 300.0
                nc.tensor.matmul(
                    ps,
                    lhsT=xT_t[dt_][:, st * 128:(st + 1) * 128],
                    rhs=wvT_t[dt_],
                    start=(dt_ == 0), stop=(dt_ == NDT - 1),
                )
                sc.pe += _mm_ns(512)
            return go

        def fin():
            nc.scalar.activation(
                out=vaug_t[st].rearrange("p (h c) -> p h c", c=65)[:, :, 0:64],
                in_=ps.rearrange("p (h c) -> p h c", c=64),
                func=mybir.ActivationFunctionType.Copy)
            sc.act = max(sc.act, sc.pe + SEM) + ACT_FIX + 512 * ACT_COL
        return [step(d) for d in range(NDT)], fin

    norm_ready = {}

    def oproj_group(st, hf):
        ps = fill_pool.tile([128, 512], F32, tag="fl", name="fl")

        def step(et):
            def go():
                nc.tensor.matmul(
                    ps,
                    lhsT=outT_t[et][:, st * 128:(st + 1) * 128],
                    rhs=woT_t[et][:, hf * 512:(hf + 1) * 512],
                    start=(et == 0), stop=(et == NET - 1),
                )
                sc.pe += _mm_ns(512)
            return go

        def fin():
            ysb = y_pool.tile([128, 512], BF16, tag="ysb", name="ysb")
            if st >= 12:
                # tail: DVE is busy with the final-slot norm chain; the
                # scalar engine has no exp left, so cast there and split
                # the writeback across three DMA queues
                nc.scalar.activation(out=ysb, in_=ps,
                                     func=mybir.ActivationFunctionType.Copy)
                sc.act = max(sc.act, sc.pe + SEM) + ACT_FIX + 512 * ACT_COL
                q = [nc.sync, nc.gpsimd, nc.scalar][(2 * st + hf) % 3]
            else:
                nc.vector.tensor_copy(ysb, ps)
                sc.dve += 650.0
                q = nc.gpsimd
            q.dma_start(
                out=y[st * 128:(st + 1) * 128, hf * 512:(hf + 1) * 512],
                in_=ysb)

        def gate():
            # reserve o-proj work for the exp-heavy late query groups: the
            # early windows have plenty of qkv filler, qg3 has nothing else
            min_slot = {0: 8, 1: 10, 2: 12, 3: 0}[st // 4]
            return (sc.slot_idx >= min_slot
                    and sc.pe >= norm_ready.get(st // 4, float("inf")))
        return [step(e) for e in range(NET)], fin, gate

    # ordered filler queue (qg-major forced-by order)
    order = []
    order += [("q", 0, 0), ("k", 0, 0), ("v", 0), ("v", 1), ("v", 2), ("v", 3)]
    for et in (1, 2, 3):
        order += [("q", et, 0), ("k", et, 0)]
    order += [("v", st) for st in range(4, 8)]
    for et in range(4):
        order += [("q", et, 1), ("k", et, 1)]
    order += [("v", st) for st in range(8, 12)]
    for et in range(4):
        order += [("k", et, 2), ("q", et, 2)]
    order += [("v", st) for st in range(12, 16)]
    for et in range(4):
        order += [("k", et, 3), ("q", et, 3)]
    for st in range(NST):
        order += [("o", st, 0), ("o", st, 1)]

    def _late(min_slot):
        return lambda: sc.slot_idx >= min_slot

    for key in order:
        if key[0] == "q":
            steps, fin = qk_group(wq_sl, qT_t, key[1], key[2])
            sc.add_group(key, steps, fin)
        elif key[0] == "k":
            steps, fin = qk_group(wk_sl, kT_t, key[1], key[2])
            sc.add_group(key, steps, fin)
        elif key[0] == "v":
            steps, fin = v_group(key[1])
            sc.add_group(key, steps, fin)
        else:
            steps, fin, gate = oproj_group(key[1], key[2])
            sc.add_group(key, steps, fin, gate)

    # ---- normalization (DRAM-bounce broadcast) ---------------------------
    def _norm_heads(qg, heads):
        h0, nh = heads[0], len(heads)
        sums = rec_pool.tile([nh, 512], F32, tag=f"sums{nh}", name="sums")
        nc.sync.dma_start(out=sums, in_=sums_dram[qg, h0:h0 + nh])
        rec = rec_pool.tile([nh, 512], F32, tag=f"rec{nh}", name="rec")
        nc.vector.reciprocal_approx_fast(out=rec, in_=sums)
        recb = rec_pool.tile([nh, 512], BF16, tag=f"recb{nh}", name="recb")
        nc.vector.tensor_copy(recb, rec)
        nc.sync.dma_start(out=rec_dram[qg, h0:h0 + nh], in_=recb)
        for h in heads:
            ti, po = h // 2, 64 * (h % 2)
            # walrus requires SBUF tensor_tensor inputs to share the start
            # partition, so land the broadcast at the same partition range
            bcs = rec_pool.tile([128, 512], BF16, tag="bcs", name="bcs")
            nc.sync.dma_start(
                out=bcs[po:po + 64, :],
                in_=rec_dram[qg, h:h + 1, :].to_broadcast([64, 512]))
            sl = outT_t[ti][po:po + 64, qg * 512:(qg + 1) * 512]
            # all-SBUF multiply: Pool engine, so the DVE queue stays clear
            # for the latency-critical masks
            nc.gpsimd.tensor_mul(sl, sl, bcs[po:po + 64, :])
        sc.dve += 1000.0
        if nh == HL:
            # only a full-qg normalize makes that qg's o-proj safe; the
            # qg3 per-pair normalizes must NOT open the gate early
            norm_ready[qg] = sc.pe + NORM_LAT

    def _stash(hp, qg, avA, avB, final):
        # stash unnormalized outputs + denominators; release av quickly
        ti = hp
        hA, hB = 2 * hp, 2 * hp + 1
        if final:
            # final pair: no attention left to hide the DRAM-bounce latency
            # behind, so normalize inline via reciprocal + PE broadcast.
            # The reciprocal chain (sums -> recip -> bc) runs on DVE+PE
            # while the big av->outT copies go on the now-idle scalar
            # engine in parallel.
            stg2 = rec_pool.tile([64, 512], F32, tag="stg2", name="stg2")
            nc.vector.memset(stg2, 1.0)
            # reciprocal chain first on DVE so bc launches early; the big
            # av->outT copies follow and overlap the bc matmul
            for av, row in ((avA, 0), (avB, 32)):
                nc.vector.tensor_copy(stg2[row:row + 1, :], av[64:65, :])
            rec2 = rec_pool.tile([64, 512], F32, tag="rec2", name="rec2")
            nc.vector.reciprocal_approx_fast(out=rec2, in_=stg2)
            recb2 = rec_pool.tile([64, 512], BF16, tag="recb2", name="recb2")
            nc.vector.tensor_copy(recb2, rec2)
            for av, po in ((avA, 0), (avB, 64)):
                nc.vector.tensor_copy(
                    outT_t[ti][po:po + 64, qg * 512:(qg + 1) * 512],
                    av[0:64, :])
            bc = av_pool.tile([128, 512], F32, tag="av", name="bc")
            nc.tensor.matmul(bc, lhsT=ones2, rhs=recb2, start=True, stop=True)
            sc.pe += _mm_ns(512)
            for po in (0, 64):
                sl = outT_t[ti][po:po + 64, qg * 512:(qg + 1) * 512]
                nc.vector.tensor_mul(sl, sl, bc[po:po + 64, :])
            sc.dve += 2500.0
            # the final-pair inline norm is the LAST normalization of qg3:
            # only now is qg3's outT fully normalized
            norm_ready[qg] = sc.pe + 1200.0
        else:
            for av, h, po in ((avA, hA, 0), (avB, hB, 64)):
                nc.vector.tensor_copy(
                    outT_t[ti][po:po + 64, qg * 512:(qg + 1) * 512],
                    av[0:64, :])
                stg = rec_pool.tile([1, 512], F32, tag="stg", name="stg",
                                    bufs=4)
                nc.vector.tensor_copy(stg, av[64:65, :])
                nc.sync.dma_start(out=sums_dram[qg, h], in_=stg)
            sc.dve += 1600.0

    # ---- global attention stream -----------------------------------------
    gkt = 0
    exp_done = []          # per global kt: estimated ACT completion
    av_ready = []          # per global kt: exp(+mask) ready for AV read

    def emit_scores_exp(hp, qg, kt):
        nonlocal gkt
        c = kt - 4 * qg           # >=0: diagonal strip index
        off = 128 * c if c > 0 else 0
        n = 512 - off
        if gkt >= 2:
            sc.pace_to(exp_done[gkt - 2] + MARGIN_BANK)
        ps = ps_pool.tile([128, 1024], F32, tag="mm", name="ps")
        for po in (0, 64):
            hoff = (po // 64) * 512
            nc.tensor.matmul(
                ps[:, hoff + off:hoff + 512],
                lhsT=kT_t[hp][po:po + 64, kt * 128:(kt + 1) * 128],
                rhs=qT_t[hp][po:po + 64, qg * 512 + off:(qg + 1) * 512],
                start=True, stop=True,
            )
        # the two K=64 matmuls land on disjoint PE row halves (h0/h64) and
        # execute concurrently; deliberately over-model the pair cost so
        # pacing keeps the PE comfortably ahead of the exp pipe (the v2
        # serial model measured fastest on HW)
        sc.pe += 2 * _mm_ns(n)
        score_done = sc.pe
        ex = ex_pool.tile([128, 1024], BF16, tag="ex", name="ex")
        if off > 0 and qg == 0:
            # qg0 banks have no full-width write history yet: a contiguous
            # [off:1024] read would cover never-written PSUM between the two
            # heads' valid ranges, so split per head
            for hoff in (0, 512):
                nc.scalar.activation(
                    out=ex[:, hoff + off:hoff + 512],
                    in_=ps[:, hoff + off:hoff + 512],
                    func=mybir.ActivationFunctionType.Exp, scale=0.125)
            sc.act = max(sc.act, score_done + SEM) + 2 * (ACT_FIX + n * ACT_COL)
        else:
            nc.scalar.activation(
                out=ex[:, off:1024], in_=ps[:, off:1024],
                func=mybir.ActivationFunctionType.Exp, scale=0.125)
            sc.act = max(sc.act, score_done + SEM) + ACT_FIX + (1024 - off) * ACT_COL
        exp_done.append(sc.act)
        ready = sc.act
        if c >= 0:
            # leading 128-query block of a diagonal strip: triangular mask
            for hoff in (0, 512):
                nc.vector.tensor_mul(ex[:, hoff + off:hoff + off + 128],
                                     ex[:, hoff + off:hoff + off + 128],
                                     tri_t)
            sc.dve = max(sc.dve, sc.act + SEM) + 2 * 180.0
            ready = sc.dve
        av_ready.append(ready + MARGIN_AV)
        gkt += 1
        return ex, off

    def emit_av(avA, avB, hp, qg, kt, ex, off, nk, gi):
        sc.force(("v", kt))
        sc.pace_to(av_ready[gi])
        for av, h in ((avA, 0), (avB, 1)):
            hh = 2 * hp + h
            nc.tensor.matmul(
                av[:, off:512],
                lhsT=vaug_t[kt][:, hh * 65:hh * 65 + 65],
                rhs=ex[:, h * 512 + off:h * 512 + 512],
                start=(kt == 0), stop=(kt == nk - 1),
                skip_group_check=True,
            )
            sc.pe += _mm_ns(512 - off)

    # qg-major slot order (interleaving qg3 earlier measured much worse)
    SEQ = [(qg, hp) for qg in range(NQG) for hp in range(HL // 2)]
    qg_done = [0] * NQG
    pending_norm = None
    for si, (qg, hp) in enumerate(SEQ):
        sc.slot_idx = si
        nk = 4 * qg + 4
        sc.force(("q", hp, qg))
        sc.force(("k", hp, qg))
        avA = av_pool.tile([65, 512], F32, tag="av", name="avA")
        avB = av_pool.tile([65, 512], F32, tag="av", name="avB")
        pending = []
        for kt in range(nk):
            gi = gkt
            ex, off = emit_scores_exp(hp, qg, kt)
            if kt == 2 and pending_norm is not None:
                # deferred: emitting the norm chain here keeps the next
                # slot's first masks from queueing behind it on the DVE
                pending_norm()
                pending_norm = None
            pending.append((kt, ex, off, gi))
            if len(pending) > 2:
                kt_, ex_, off_, gi_ = pending.pop(0)
                emit_av(avA, avB, hp, qg, kt_, ex_, off_, nk, gi_)
        for kt_, ex_, off_, gi_ in pending:
            emit_av(avA, avB, hp, qg, kt_, ex_, off_, nk, gi_)
        _stash(hp, qg, avA, avB, final=(qg == 3 and hp == 3))
        qg_done[qg] += 1
        if qg < 3:
            if qg_done[qg] == 4:
                pending_norm = (lambda q_=qg:
                                _norm_heads(q_, list(range(HL))))
        elif hp < 3:
            pending_norm = (lambda q_=qg, h_=hp:
                            _norm_heads(q_, [2 * h_, 2 * h_ + 1]))

    sc.slot_idx = 16
    sc.drain()
    sched_stats.update(pe=sc.pe, act=sc.act, dve=sc.dve, stalled=sc.stalled,
                       stall_log=sc.stall_log)


def _build():
    nc = bacc.Bacc("TRN2", target_bir_lowering=False, debug=False,
                   num_devices=NCORES)
    xT = nc.dram_tensor("xT", [D, S], BF16, kind="ExternalInput").ap()
    wqT = nc.dram_tensor("wqT", [128, NDT * E], BF16, kind="ExternalInput").ap()
    wkT = nc.dram_tensor("wkT", [128, NDT * E], BF16, kind="ExternalInput").ap()
    wvT = nc.dram_tensor("wvT", [128, NDT * E], BF16, kind="ExternalInput").ap()
    woT = nc.dram_tensor("woT", [128, NET * D], BF16, kind="ExternalInput").ap()
    tri = nc.dram_tensor("tri", [128, 128], BF16,
                         kind="ExternalInput").ap()
    y = nc.dram_tensor("y", [S, D], BF16, kind="ExternalOutput").ap()
    with tile.TileContext(nc) as tc:
        _mhsa_kernel(tc, y, xT, wqT, wkT, wvT, woT, tri)
    nc.compile()
    return nc


def get_compiled():
    global _compiled
    if _compiled is None:
        _compiled = _build()
    return _compiled


def _make_tri():
    # tri[i, j] keeps key i <= query j within a 128-aligned diagonal block
    col = np.arange(128)
    return (col[None, :] >= col[:, None]).astype(bf16)


def _pack_dt(w, ncols):
    # [NT*128, ncols] -> [128, NT*ncols] with dt-major column blocks
    nt = w.shape[0] // 128
    return np.ascontiguousarray(
        w.reshape(nt, 128, ncols).transpose(1, 0, 2).reshape(128, nt * ncols))


def _pack_etdt(w):
    # [1024, 512] -> [128, et*1024 + dt*128 + c] (et-major, dt blocks)
    return np.ascontiguousarray(
        w.reshape(NDT, 128, NET, 128).transpose(1, 2, 0, 3).reshape(128, NDT * E))


def make_in_maps(inputs):
    x = np.asarray(inputs["in_features"], dtype=np.float32)
    w_q = np.asarray(inputs["w_q"], dtype=np.float32)
    w_k = np.asarray(inputs["w_k"], dtype=np.float32)
    w_v = np.asarray(inputs["w_v"], dtype=np.float32)
    w_o = np.asarray(inputs["w_o"], dtype=np.float32)
    tri = _make_tri()
    in_maps = []
    for c in range(NCORES):
        b, hg = divmod(c, 2)
        es = slice(hg * E, (hg + 1) * E)
        in_maps.append({
            "xT": x[b].T.astype(bf16),
            "wqT": _pack_etdt(w_q[es, :].T.astype(bf16)),
            "wkT": _pack_etdt(w_k[es, :].T.astype(bf16)),
            "wvT": _pack_dt(w_v[es, :].T.astype(bf16), E),
            "woT": _pack_dt(w_o[:, es].T.astype(bf16), D),
            "tri": tri,
        })
    return in_maps


def kernel(**inputs):
    global last_results
    nc = get_compiled()
    in_maps = make_in_maps(inputs)
    res = run_bass_kernel_spmd(nc, in_maps, list(range(NCORES)))
    last_results = res
    y = np.zeros((B, S, D), dtype=np.float32)
    for c in range(NCORES):
        y[c // 2] += np.asarray(res.results[c]["y"], dtype=np.float32)
    return y


# revision 68
# speedup vs baseline: 1.0070x; 1.0070x over previous
"""Multi-head self-attention (causal) Trainium2 Bass/Tile kernel, 8-way SPMD.

Sharding: data-parallel over batch (4) x tensor-parallel over heads (2 groups
of 8 heads).  Core c handles batch c//2, head-group c%2.  Each core computes
q/k/v projections for its 512 local features, causal attention for its 8
heads, and a partial o-projection (contraction over its 512 features of the
attention output) giving a full-shape [S, D] partial that the host sums per
batch pair.

All matmul operands are bf16 (fp32 PSUM accumulation); softmax runs without
max-subtraction (scores ~ N(0,1) after the 1/8 scale, no overflow risk), with
exp on the scalar engine and the row-sum folded into the AV matmul via a ones
column appended to V.  Host pre-transposes inputs so no on-chip transposes
are needed:
  qT[e,s]  = wqT.T @ xT        (lhsT=wqT[d,e], rhs=xT[d,s])
  scoresT[sk,sq] = kT.T @ qT   (lhsT=kT[dk,sk], rhs=qT[dk,sq], K=64)
  avT[dk+1,sq]   = vaug.T @ expT  (lhsT=vaug[sk,65], rhs=expT[sk,sq])
  y[s,e]   = outT.T @ woT      (lhsT=outT[d,s], rhs=woT[d,e])

Scheduling: a single global stream over (query-group, head-pair, key-tile)
"attention steps", software-pipelined with the projection / o-projection
matmuls broken into single-matmul filler quanta.  Cost-model cursors for the
PE / ACT / DVE engines decide when the attention stream would stall on the
scalar engine's exp (score-PSUM bank rotation), and filler is injected to
cover the wait; the pair of K=64 score matmuls lands on disjoint PE row
halves (h0/h64) and executes concurrently, and the pacing deliberately
over-budgets them so the PE stays ahead of the exp pipe.  Causality is
exploited at 128-column granularity: diagonal key strips only compute
scores/exp/AV for the query range they can see, with one shared [128,128]
lower-triangular mask for the leading block.  o-proj groups are reserved
(slot gates) for the exp-heavy late query groups, where they are the only
filler left.  Weights are packed host-side so each matrix loads in a few
large DMAs ordered by first need (wq/wk et-major); xT streams token-block-
major across the gpsimd+scalar DMA queues.  y is written back as bf16
partials summed on the host.
"""

from contextlib import ExitStack

import numpy as np
import ml_dtypes

import concourse.bass as bass
import concourse.tile as tile
from concourse import bacc, mybir
from concourse._compat import with_exitstack
from concourse.bass_utils import run_bass_kernel_spmd

B, S, D, H = 4, 2048, 1024, 16
DK = D // H          # 64
E = 512              # local features per core (8 heads)
HL = 8               # local heads
NCORES = 8
NDT = D // 128       # 8 d-tiles
NET = E // 128       # 4 e-tiles
NST = S // 128       # 16 s-tiles
NQG = S // 512       # 4 query groups

F32 = mybir.dt.float32
BF16 = mybir.dt.bfloat16
bf16 = ml_dtypes.bfloat16

# --- scheduler cost model (ns) ------------------------------------------
CYC = 1e9 / 2.4e9          # PE ns per streamed column (bf16)
PE_OVH = 8.0               # per-matmul PE overhead (trace: 220ns/512-col)
ACT_FIX = 220.0            # per-activation fixed cost (trace fit)
ACT_COL = 0.91             # ACT ns per free column (trace fit)
DVE_COL = 1e9 / 0.96e9     # DVE ns per free column (f32 path)
SEM = 100.0                # cross-engine semaphore latency
WARMUP = 22                # PE clock-ramp matmuls at kernel start
MARGIN_BANK = 120.0        # pacing margin for score-PSUM bank reuse
MARGIN_AV = 80.0           # pacing margin for AV reading exp(+mask)
NORM_LAT = 6000.0          # DRAM-bounce + Pool-mul latency for normalize

_compiled = None
last_results = None  # test harness introspection
sched_stats = {}


def _mm_ns(n):
    return n * CYC + PE_OVH


class _Sched:
    """Engine-cursor bookkeeping + filler queue."""

    def __init__(self):
        self.pe = 0.0
        self.act = 0.0
        self.dve = 0.0
        self.groups = []          # [steps:list, fin:callable, gate:callable]
        self.key_idx = {}
        self.cur = 0
        self.active = None
        self.stalled = 0.0
        self.slot_idx = 0
        self.stall_log = []

    # -- filler queue ----------------------------------------------------
    # Strictly ordered groups; pacing may jump over gated (not-yet-eligible)
    # groups, but only ONE jump-ahead group may be in flight at a time so
    # PSUM fill-buffer rotation never interleaves two accumulations.
    def add_group(self, key, steps, fin=None, gate=None):
        self.key_idx[key] = len(self.groups)
        self.groups.append([list(steps), fin, gate])

    def _emit_from(self, idx):
        steps, fin, _ = self.groups[idx]
        if steps:
            steps.pop(0)()
            if not steps and fin is not None:
                fin()
                self.groups[idx][1] = None
            return True
        if fin is not None:
            fin()
            self.groups[idx][1] = None
            return True
        return False

    def _done(self, idx):
        g = self.groups[idx]
        return not g[0] and g[1] is None

    def emit_one(self, pacing=False):
        while self.cur < len(self.groups) and self._done(self.cur):
            self.cur += 1
        if self.active is not None:
            idx = self.active
            if self._done(idx):
                self.active = None
            else:
                self._emit_from(idx)
                if self._done(idx):
                    self.active = None
                return True
        if self.cur >= len(self.groups):
            return False
        steps, fin, gate = self.groups[self.cur]
        if not pacing or gate is None or gate():
            return self._emit_from(self.cur)
        # cur is gated: jump ahead to the first eligible group
        for idx in range(self.cur + 1, len(self.groups)):
            g = self.groups[idx]
            if self._done(idx):
                continue
            if g[2] is not None and not g[2]():
                continue
            self.active = idx
            self._emit_from(idx)
            if self._done(idx):
                self.active = None
            return True
        return False

    def force(self, key):
        idx = self.key_idx[key]
        while self.active is not None:
            self.emit_one()
        while self.cur <= idx and not self._done(idx):
            if not self.emit_one():
                return

    def drain(self):
        while self.emit_one():
            pass

    def pace_to(self, target):
        while self.pe + 1e-9 < target:
            if not self.emit_one(pacing=True):
                self.stalled += target - self.pe
                self.stall_log.append((self.slot_idx, round(self.pe),
                                       round(target - self.pe)))
                self.pe = target
                break


@with_exitstack
def _mhsa_kernel(ctx: ExitStack, tc: tile.TileContext, y, xT, wqT, wkT, wvT,
                 woT, tri):
    nc = tc.nc
    sc = _Sched()

    consts = ctx.enter_context(tc.tile_pool(name="consts", bufs=1))
    ex_pool = ctx.enter_context(tc.tile_pool(name="ex", bufs=6))
    rec_pool = ctx.enter_context(tc.tile_pool(name="rec", bufs=2))
    y_pool = ctx.enter_context(tc.tile_pool(name="ysb", bufs=3))
    ps_pool = ctx.enter_context(tc.tile_pool(name="psmm", bufs=2, space="PSUM"))
    fill_pool = ctx.enter_context(tc.tile_pool(name="psfl", bufs=2, space="PSUM"))
    av_pool = ctx.enter_context(tc.tile_pool(name="psav", bufs=2, space="PSUM"))

    def ctile(shape, dt_, tg):
        return consts.tile(shape, dt_, tag=tg, name=tg)

    # ---- persistent SBUF tiles -------------------------------------------
    # weights are packed host-side as [128, NDT*cols] (dt-major columns) so
    # each matrix loads with a single dma_start
    xT_t = [ctile([128, S], BF16, f"xT{i}") for i in range(NDT)]
    wq_all = ctile([128, NDT * E], BF16, "wq_all")
    wk_all = ctile([128, NDT * E], BF16, "wk_all")
    wv_all = ctile([128, NDT * E], BF16, "wv_all")
    wo_all = ctile([128, NET * D], BF16, "wo_all")
    qT_t = [ctile([128, S], BF16, f"qT{i}") for i in range(NET)]
    kT_t = [ctile([128, S], BF16, f"kT{i}") for i in range(NET)]
    vaug_t = [ctile([128, HL * (DK + 1)], BF16, f"vaug{i}") for i in range(NST)]
    outT_t = [ctile([128, S], BF16, f"outT{i}") for i in range(NET)]
    tri_t = ctile([128, 128], BF16, "tri")

    # wq/wk are packed ET-MAJOR host-side ([128, et*1024 + dt*128 + c]) so
    # the first head-pair's weights arrive as one small early DMA; wv/wo
    # stay dt-major (their consumers need all of them anyway)
    def wq_sl(et, dt):
        return wq_all[:, et * 1024 + dt * 128:et * 1024 + (dt + 1) * 128]

    def wk_sl(et, dt):
        return wk_all[:, et * 1024 + dt * 128:et * 1024 + (dt + 1) * 128]

    wvT_t = [wv_all[:, i * E:(i + 1) * E] for i in range(NDT)]
    woT_t = [wo_all[:, i * D:(i + 1) * D] for i in range(NET)]

    # ---- input DMA, ordered by first need ---------------------------------
    nc.sync.dma_start(out=tri_t, in_=tri)
    for et in range(NET):
        nc.sync.dma_start(out=wq_all[:, et * 1024:(et + 1) * 1024],
                          in_=wqT[:, et * 1024:(et + 1) * 1024])
        nc.sync.dma_start(out=wk_all[:, et * 1024:(et + 1) * 1024],
                          in_=wkT[:, et * 1024:(et + 1) * 1024])
        if et == 0:
            nc.sync.dma_start(out=wv_all, in_=wvT)
    # warm memset goes on gpsimd so it lands before anything else that
    # engine issues; the DVE queue is kept clear for latency-critical masks
    warm = ctile([128, 512], BF16, "warm")
    nc.gpsimd.memset(warm, 0.0)
    # xT token-block tb0 first (gates the first scores), split across the
    # gpsimd and scalar DMA queues with even/odd dt interleaved so the two
    # transfer streams deliver tiles in accumulation order
    for tb in range(2):
        for i in range(NDT):
            q = nc.gpsimd if i % 2 == 0 else nc.scalar
            q.dma_start(out=xT_t[i][:, tb * 512:(tb + 1) * 512],
                        in_=xT[i * 128:(i + 1) * 128, tb * 512:(tb + 1) * 512])
    for i in range(NDT):
        nc.gpsimd.dma_start(
            out=xT_t[i][:, 1024:2048],
            in_=xT[i * 128:(i + 1) * 128, 1024:2048])
    nc.sync.dma_start(out=wo_all, in_=woT)

    # ---- PE warm-up: HAM starts throttled at 1.2 GHz and needs ~3us of
    # sustained matmul activity to release; burn idle DMA-wait time at
    # kernel start so the first real matmuls run at full clock.  One
    # accumulation group: no inter-matmul completion semaphores, so it
    # actually runs back-to-back.  Emitted before the filler groups so the
    # fill-pool rotation order matches emission order.
    wps = fill_pool.tile([128, 512], F32, tag="fl", name="fl")
    for i in range(WARMUP):
        nc.tensor.matmul(wps, lhsT=warm[:, 0:128], rhs=warm,
                         start=(i == 0), stop=(i == WARMUP - 1))
        sc.pe += _mm_ns(512)

    # Softmax denominators bounce through DRAM: DVE can only write at
    # 32-aligned base partitions, and SBUF APs cannot have a step-0
    # partition dim (needed for the broadcast) — DRAM APs can do both.
    sums_dram = nc.dram_tensor("sums_bounce", [NQG, HL, 512], F32).ap()
    rec_dram = nc.dram_tensor("rec_bounce", [NQG, HL, 512], BF16).ap()

    # ones2: selector for the final pair's reciprocal broadcast matmul —
    # bc[j, :] = recb2[0, :] for j<64 (head A) and recb2[32, :] for j>=64
    # (head B).  Rows 0/32 because the DVE can only write at 32-aligned
    # partitions; K padded to 64 (a K=33 matmul wedges the exec unit).
    ones2 = ctile([64, 128], BF16, "ones2")
    nc.vector.memset(ones2, 0.0)
    nc.vector.memset(ones2[0:1, 0:64], 1.0)
    nc.vector.memset(ones2[32:33, 64:128], 1.0)

    # ---- filler groups ----------------------------------------------------
    def qk_group(wsl, dst, et, tb):
        ps = fill_pool.tile([128, 512], F32, tag="fl", name="fl")

        def step(dt_):
            def go():
                nc.tensor.matmul(
                    ps,
                    lhsT=wsl(et, dt_),
                    rhs=xT_t[dt_][:, tb * 512:(tb + 1) * 512],
                    start=(dt_ == 0), stop=(dt_ == NDT - 1),
                )
                sc.pe += _mm_ns(512)
            return go

        def fin():
            # Pool can't read PSUM; scalar Copy keeps the DVE queue clear
            # for the latency-critical masks
            nc.scalar.activation(
                out=dst[et][:, tb * 512:(tb + 1) * 512], in_=ps,
                func=mybir.ActivationFunctionType.Copy)
            sc.act = max(sc.act, sc.pe + SEM) + ACT_FIX + 512 * ACT_COL
        return [step(d) for d in range(NDT)], fin

    def v_group(st):
        ps = fill_pool.tile([128, 512], F32, tag="fl", name="fl")

        def step(dt_):
            def go():
                if dt_ == 0:
                    # lazy memset: keeps the DVE queue clear until needed
                    nc.vector.memset(vaug_t[st], 1.0)
                    sc.dve += 300.0
                nc.tensor.matmul(
                    ps,
                    lhsT=xT_t[dt_][:, st * 128:(st + 1) * 128],
                    rhs=wvT_t[dt_],
                    start=(dt_ == 0), stop=(dt_ == NDT - 1),
                )
                sc.pe += _mm_ns(512)
            return go

        def fin():
            nc.scalar.activation(
                out=vaug_t[st].rearrange("p (h c) -> p h c", c=65)[:, :, 0:64],
                in_=ps.rearrange("p (h c) -> p h c", c=64),
                func=mybir.ActivationFunctionType.Copy)
            sc.act = max(sc.act, sc.pe + SEM) + ACT_FIX + 512 * ACT_COL
        return [step(d) for d in range(NDT)], fin

    norm_ready = {}

    def oproj_group(st, hf):
        ps = fill_pool.tile([128, 512], F32, tag="fl", name="fl")

        def step(et):
            def go():
                nc.tensor.matmul(
                    ps,
                    lhsT=outT_t[et][:, st * 128:(st + 1) * 128],
                    rhs=woT_t[et][:, hf * 512:(hf + 1) * 512],
                    start=(et == 0), stop=(et == NET - 1),
                )
                sc.pe += _mm_ns(512)
            return go

        def fin():
            ysb = y_pool.tile([128, 512], BF16, tag="ysb", name="ysb")
            if st >= 12:
                # tail: DVE is busy with the final-slot norm chain; the
                # scalar engine has no exp left, so cast there and split
                # the writeback across three DMA queues
                nc.scalar.activation(out=ysb, in_=ps,
                                     func=mybir.ActivationFunctionType.Copy)
                sc.act = max(sc.act, sc.pe + SEM) + ACT_FIX + 512 * ACT_COL
                q = [nc.sync, nc.gpsimd, nc.scalar][(2 * st + hf) % 3]
            else:
                nc.vector.tensor_copy(ysb, ps)
                sc.dve += 650.0
                q = nc.gpsimd
            q.dma_start(
                out=y[st * 128:(st + 1) * 128, hf * 512:(hf + 1) * 512],
                in_=ysb)

        def gate():
            # reserve o-proj work for the exp-heavy late query groups: the
            # early windows have plenty of qkv filler, qg3 has nothing else
            min_slot = {0: 8, 1: 10, 2: 12, 3: 0}[st // 4]
            return (sc.slot_idx >= min_slot
                    and sc.pe >= norm_ready.get(st // 4, float("inf")))
        return [step(e) for e in range(NET)], fin, gate

    # ordered filler queue (qg-major forced-by order)
    order = []
    order += [("q", 0, 0), ("k", 0, 0), ("v", 0), ("v", 1), ("v", 2), ("v", 3)]
    for et in (1, 2, 3):
        order += [("q", et, 0), ("k", et, 0)]
    order += [("v", st) for st in range(4, 8)]
    for et in range(4):
        order += [("q", et, 1), ("k", et, 1)]
    order += [("v", st) for st in range(8, 12)]
    for et in range(4):
        order += [("k", et, 2), ("q", et, 2)]
    order += [("v", st) for st in range(12, 16)]
    for et in range(4):
        order += [("k", et, 3), ("q", et, 3)]
    for st in range(NST):
        order += [("o", st, 0), ("o", st, 1)]

    def _late(min_slot):
        return lambda: sc.slot_idx >= min_slot

    for key in order:
        if key[0] == "q":
            steps, fin = qk_group(wq_sl, qT_t, key[1], key[2])
            sc.add_group(key, steps, fin)
        elif key[0] == "k":
            steps, fin = qk_group(wk_sl, kT_t, key[1], key[2])
            sc.add_group(key, steps, fin)
        elif key[0] == "v":
            steps, fin = v_group(key[1])
            sc.add_group(key, steps, fin)
        else:
            steps, fin, gate = oproj_group(key[1], key[2])
            sc.add_group(key, steps, fin, gate)

    # ---- normalization (DRAM-bounce broadcast) ---------------------------
    def _norm_heads(qg, heads):
        h0, nh = heads[0], len(heads)
        sums = rec_pool.tile([nh, 512], F32, tag=f"sums{nh}", name="sums")
        nc.sync.dma_start(out=sums, in_=sums_dram[qg, h0:h0 + nh])
        rec = rec_pool.tile([nh, 512], F32, tag=f"rec{nh}", name="rec")
        nc.vector.reciprocal_approx_fast(out=rec, in_=sums)
        recb = rec_pool.tile([nh, 512], BF16, tag=f"recb{nh}", name="recb")
        nc.vector.tensor_copy(recb, rec)
        nc.sync.dma_start(out=rec_dram[qg, h0:h0 + nh], in_=recb)
        for h in heads:
            ti, po = h // 2, 64 * (h % 2)
            # walrus requires SBUF tensor_tensor inputs to share the start
            # partition, so land the broadcast at the same partition range
            bcs = rec_pool.tile([128, 512], BF16, tag="bcs", name="bcs")
            nc.sync.dma_start(
                out=bcs[po:po + 64, :],
                in_=rec_dram[qg, h:h + 1, :].to_broadcast([64, 512]))
            sl = outT_t[ti][po:po + 64, qg * 512:(qg + 1) * 512]
            # all-SBUF multiply: Pool engine, so the DVE queue stays clear
            # for the latency-critical masks
            nc.gpsimd.tensor_mul(sl, sl, bcs[po:po + 64, :])
        sc.dve += 1000.0
        if nh == HL:
            # only a full-qg normalize makes that qg's o-proj safe; the
            # qg3 per-pair normalizes must NOT open the gate early
            norm_ready[qg] = sc.pe + NORM_LAT

    def _stash(hp, qg, avA, avB, final):
        # stash unnormalized outputs + denominators; release av quickly
        ti = hp
        hA, hB = 2 * hp, 2 * hp + 1
        if final:
            # final pair: no attention left to hide the DRAM-bounce latency
            # behind, so normalize inline via reciprocal + PE broadcast.
            # The reciprocal chain (sums -> recip -> bc) runs on DVE+PE
            # while the big av->outT copies go on the now-idle scalar
            # engine in parallel.
            stg2 = rec_pool.tile([64, 512], F32, tag="stg2", name="stg2")
            nc.vector.memset(stg2, 1.0)
            # reciprocal chain first on DVE so bc launches early; the big
            # av->outT copies follow and overlap the bc matmul
            for av, row in ((avA, 0), (avB, 32)):
                nc.vector.tensor_copy(stg2[row:row + 1, :], av[64:65, :])
            rec2 = rec_pool.tile([64, 512], F32, tag="rec2", name="rec2")
            nc.vector.reciprocal_approx_fast(out=rec2, in_=stg2)
            recb2 = rec_pool.tile([64, 512], BF16, tag="recb2", name="recb2")
            nc.vector.tensor_copy(recb2, rec2)
            for av, po in ((avA, 0), (avB, 64)):
                nc.vector.tensor_copy(
                    outT_t[ti][po:po + 64, qg * 512:(qg + 1) * 512],
                    av[0:64, :])
            bc = av_pool.tile([128, 512], F32, tag="av", name="bc")
            nc.tensor.matmul(bc, lhsT=ones2, rhs=recb2, start=True, stop=True)
            sc.pe += _mm_ns(512)
            for po in (0, 64):
                sl = outT_t[ti][po:po + 64, qg * 512:(qg + 1) * 512]
                nc.vector.tensor_mul(sl, sl, bc[po:po + 64, :])
            sc.dve += 2500.0
            # the final-pair inline norm is the LAST normalization of qg3:
            # only now is qg3's outT fully normalized
            norm_ready[qg] = sc.pe + 1200.0
        else:
            for av, h, po in ((avA, hA, 0), (avB, hB, 64)):
                nc.vector.tensor_copy(
                    outT_t[ti][po:po + 64, qg * 512:(qg + 1) * 512],
                    av[0:64, :])
                stg = rec_pool.tile([1, 512], F32, tag="stg", name="stg",
                                    bufs=4)
                nc.vector.tensor_copy(stg, av[64:65, :])
                nc.sync.dma_start(out=sums_dram[qg, h], in_=stg)
            sc.dve += 1600.0

    # ---- global attention stream -----------------------------------------
    gkt = 0
    exp_done = []          # per global kt: estimated ACT completion
    av_ready = []          # per global kt: exp(+mask) ready for AV read

    def emit_scores_exp(hp, qg, kt):
        nonlocal gkt
        c = kt - 4 * qg           # >=0: diagonal strip index
        off = 128 * c if c > 0 else 0
        n = 512 - off
        if gkt >= 2:
            sc.pace_to(exp_done[gkt - 2] + MARGIN_BANK)
        ps = ps_pool.tile([128, 1024], F32, tag="mm", name="ps")
        for po in (0, 64):
            hoff = (po // 64) * 512
            nc.tensor.matmul(
                ps[:, hoff + off:hoff + 512],
                lhsT=kT_t[hp][po:po + 64, kt * 128:(kt + 1) * 128],
                rhs=qT_t[hp][po:po + 64, qg * 512 + off:(qg + 1) * 512],
                start=True, stop=True,
            )
        # the two K=64 matmuls land on disjoint PE row halves (h0/h64) and
        # execute concurrently; deliberately over-model the pair cost so
        # pacing keeps the PE comfortably ahead of the exp pipe (the v2
        # serial model measured fastest on HW)
        sc.pe += 2 * _mm_ns(n)
        score_done = sc.pe
        ex = ex_pool.tile([128, 1024], BF16, tag="ex", name="ex")
        if off > 0 and qg == 0:
            # qg0 banks have no full-width write history yet: a contiguous
            # [off:1024] read would cover never-written PSUM between the two
            # heads' valid ranges, so split per head
            for hoff in (0, 512):
                nc.scalar.activation(
                    out=ex[:, hoff + off:hoff + 512],
                    in_=ps[:, hoff + off:hoff + 512],
                    func=mybir.ActivationFunctionType.Exp, scale=0.125)
            sc.act = max(sc.act, score_done + SEM) + 2 * (ACT_FIX + n * ACT_COL)
        else:
            nc.scalar.activation(
                out=ex[:, off:1024], in_=ps[:, off:1024],
                func=mybir.ActivationFunctionType.Exp, scale=0.125)
            sc.act = max(sc.act, score_done + SEM) + ACT_FIX + (1024 - off) * ACT_COL
        exp_done.append(sc.act)
        ready = sc.act
        if c >= 0:
            # leading 128-query block of a diagonal strip: triangular mask
            for hoff in (0, 512):
                nc.vector.tensor_mul(ex[:, hoff + off:hoff + off + 128],
                                     ex[:, hoff + off:hoff + off + 128],
                                     tri_t)
            sc.dve = max(sc.dve, sc.act + SEM) + 2 * 180.0
            ready = sc.dve
        av_ready.append(ready + MARGIN_AV)
        gkt += 1
        return ex, off

    def emit_av(avA, avB, hp, qg, kt, ex, off, nk, gi):
        sc.force(("v", kt))
        sc.pace_to(av_ready[gi])
        for av, h in ((avA, 0), (avB, 1)):
            hh = 2 * hp + h
            nc.tensor.matmul(
                av[:, off:512],
                lhsT=vaug_t[kt][:, hh * 65:hh * 65 + 65],
                rhs=ex[:, h * 512 + off:h * 512 + 512],
                start=(kt == 0), stop=(kt == nk - 1),
                skip_group_check=True,
            )
            sc.pe += _mm_ns(512 - off)

    # qg-major slot order (interleaving qg3 earlier measured much worse)
    SEQ = [(qg, hp) for qg in range(NQG) for hp in range(HL // 2)]
    qg_done = [0] * NQG
    pending_norm = None
    for si, (qg, hp) in enumerate(SEQ):
        sc.slot_idx = si
        nk = 4 * qg + 4
        sc.force(("q", hp, qg))
        sc.force(("k", hp, qg))
        avA = av_pool.tile([65, 512], F32, tag="av", name="avA")
        avB = av_pool.tile([65, 512], F32, tag="av", name="avB")
        pending = []
        for kt in range(nk):
            gi = gkt
            ex, off = emit_scores_exp(hp, qg, kt)
            if kt == 2 and pending_norm is not None:
                # deferred: emitting the norm chain here keeps the next
                # slot's first masks from queueing behind it on the DVE
                pending_norm()
                pending_norm = None
            pending.append((kt, ex, off, gi))
            if len(pending) > 2:
                kt_, ex_, off_, gi_ = pending.pop(0)
                emit_av(avA, avB, hp, qg, kt_, ex_, off_, nk, gi_)
        for kt_, ex_, off_, gi_ in pending:
            emit_av(avA, avB, hp, qg, kt_, ex_, off_, nk, gi_)
        _stash(hp, qg, avA, avB, final=(qg == 3 and hp == 3))
        qg_done[qg] += 1
        if qg < 3:
            if qg_done[qg] == 4:
                pending_norm = (lambda q_=qg:
                                _norm_heads(q_, list(range(HL))))
        elif hp < 3:
            pending_norm = (lambda q_=qg, h_=hp:
                            _norm_heads(q_, [2 * h_, 2 * h_ + 1]))

    sc.slot_idx = 16
    sc.drain()
    sched_stats.update(pe=sc.pe, act=sc.act, dve=sc.dve, stalled=sc.stalled,
                       stall_log=sc.stall_log)


def _build():
    nc = bacc.Bacc("TRN2", target_bir_lowering=False, debug=False,
                   num_devices=NCORES)
    xT = nc.dram_tensor("xT", [D, S], BF16, kind="ExternalInput").ap()
    wqT = nc.dram_tensor("wqT", [128, NDT * E], BF16, kind="ExternalInput").ap()
    wkT = nc.dram_tensor("wkT", [128, NDT * E], BF16, kind="ExternalInput").ap()
    wvT = nc.dram_tensor("wvT", [128, NDT * E], BF16, kind="ExternalInput").ap()
    woT = nc.dram_tensor("woT", [128, NET * D], BF16, kind="ExternalInput").ap()
    tri = nc.dram_tensor("tri", [128, 128], BF16,
                         kind="ExternalInput").ap()
    y = nc.dram_tensor("y", [S, D], BF16, kind="ExternalOutput").ap()
    with tile.TileContext(nc) as tc:
        _mhsa_kernel(tc, y, xT, wqT, wkT, wvT, woT, tri)
    nc.compile()
    return nc


def get_compiled():
    global _compiled
    if _compiled is None:
        _compiled = _build()
    return _compiled


def _make_tri():
    # tri[i, j] keeps key i <= query j within a 128-aligned diagonal block
    col = np.arange(128)
    return (col[None, :] >= col[:, None]).astype(bf16)


def _pack_dt(w, ncols):
    # [NT*128, ncols] -> [128, NT*ncols] with dt-major column blocks
    nt = w.shape[0] // 128
    return np.ascontiguousarray(
        w.reshape(nt, 128, ncols).transpose(1, 0, 2).reshape(128, nt * ncols))


def _pack_etdt(w):
    # [1024, 512] -> [128, et*1024 + dt*128 + c] (et-major, dt blocks)
    return np.ascontiguousarray(
        w.reshape(NDT, 128, NET, 128).transpose(1, 2, 0, 3).reshape(128, NDT * E))


def make_in_maps(inputs):
    x = np.asarray(inputs["in_features"], dtype=np.float32)
    w_q = np.asarray(inputs["w_q"], dtype=np.float32)
    w_k = np.asarray(inputs["w_k"], dtype=np.float32)
    w_v = np.asarray(inputs["w_v"], dtype=np.float32)
    w_o = np.asarray(inputs["w_o"], dtype=np.float32)
    tri = _make_tri()
    in_maps = []
    for c in range(NCORES):
        b, hg = divmod(c, 2)
        es = slice(hg * E, (hg + 1) * E)
        in_maps.append({
            "xT": x[b].T.astype(bf16),
            "wqT": _pack_etdt(w_q[es, :].T.astype(bf16)),
            "wkT": _pack_etdt(w_k[es, :].T.astype(bf16)),
            "wvT": _pack_dt(w_v[es, :].T.astype(bf16), E),
            "woT": _pack_dt(w_o[:, es].T.astype(bf16), D),
            "tri": tri,
        })
    return in_maps


def kernel(**inputs):
    global last_results
    nc = get_compiled()
    in_maps = make_in_maps(inputs)
    res = run_bass_kernel_spmd(nc, in_maps, list(range(NCORES)))
    last_results = res
    y = np.zeros((B, S, D), dtype=np.float32)
    for c in range(NCORES):
        y[c // 2] += np.asarray(res.results[c]["y"], dtype=np.float32)
    return y
